# revision 10
# baseline (speedup 1.0000x reference)
"""Trainium2 Bass kernel for a transformer encoder layer.

B=4, S=2048, D=1024, H=16 heads (HD=64), PF=4096, fp32 I/O.

Sharding: 8 cores, core c handles batch c//2, query seq-half c%2 (1024
tokens). Each core computes K/V over its batch's full 2048-token sequence
(duplicated within the pair; ~12% extra flops) so no collectives are needed.

Dataflow (per core) keeps attention transposed so no P-matrix transpose is
ever required:
  srcT = src^T (TensorE transposes)
  QT = Wq^T srcT_q, KT = Wk^T srcT (f32r), V = srcT^T Wv (bf16, with a
      per-head ones column appended so PV^T also yields softmax denominators)
  S^T = K Q^T (f32r, contraction=HD), expS = exp(S^T/8) (bf16)
  x^T = V_aug^T expS (bf16, contraction=Sk) -> normalize by denom row
  attn_out = (x^T)^T Wo, + src + bo, LN1 -> src1 (+ src1^T)
  h^T = relu(W1^T src1^T + bf1) (bf16), ff = (h^T)^T W2 + bf2
  out = LN2(src1 + ff)
"""

import numpy as np

D = 1024
S2 = 2048
SQ = 1024
PF = 4096
H = 16
HD = 64
DK = D // 128          # 8 contraction chunks over D
PFK = PF // 128        # 32 contraction chunks over PF
NG = 4                 # head groups
HPG = H // NG          # 4 heads per group
GW = HPG * HD          # 256 output dims per group
SCALE = 1.0 / 8.0      # 1/sqrt(HD)
EPS = 1e-5
N_CORES = 8

_CACHE = {}


def _build():
    import concourse.bass as bass
    import concourse.mybir as mybir
    import concourse.tile as tile
    from concourse import bacc
    from concourse.masks import make_identity

    f32 = mybir.dt.float32
    f32r = mybir.dt.float32r
    bf16 = mybir.dt.bfloat16
    AF = mybir.ActivationFunctionType
    ALU = mybir.AluOpType

    nc = bacc.Bacc("TRN2", target_bir_lowering=False, debug=False, num_devices=N_CORES)

    def din(name, shape, dt=f32):
        return nc.dram_tensor(name, shape, dt, kind="ExternalInput")

    src_q = din("src_q", [SQ, D])     # this core's query tokens
    src_o = din("src_o", [SQ, D])     # the other half of the sequence
    Wq = din("Wq", [D, D], f32r)
    Wk = din("Wk", [D, D], f32r)
    Wv = din("Wv", [D, D], f32r)
    Wo = din("Wo", [D, D], f32r)
    W1 = din("W1", [D, PF], f32r)
    W2 = din("W2", [PF, D])
    bq = din("bq", [D])
    bk = din("bk", [D])
    bv = din("bv", [D])
    bo = din("bo", [D])
    bf1 = din("bf1", [PF])
    bf2 = din("bf2", [D])
    g1 = din("g1", [D])
    b1 = din("b1", [D])
    g2 = din("g2", [D])
    b2 = din("b2", [D])
    out = nc.dram_tensor("out", [SQ, D], f32, kind="ExternalOutput")

    xT_dram = nc.dram_tensor("xT_scratch", [D, SQ], f32r)
    src1_dram = nc.dram_tensor("src1_scratch", [SQ, D], f32)

    def bc_ap(vec, n):
        # [n] dram vector broadcast to [128, n]
        return bass.AP(tensor=vec, offset=0, ap=[[0, 128], [1, n]])

    def col_ap(vec, m):
        # [m*128] dram vector viewed as [128, m] columns
        return bass.AP(tensor=vec, offset=0, ap=[[1, 128], [128, m]])

    with tile.TileContext(nc) as tc:
        import contextlib

        with contextlib.ExitStack() as ctx:
            consts = ctx.enter_context(tc.tile_pool(name="consts", bufs=1))
            psum = ctx.enter_context(tc.tile_pool(name="psum", bufs=1, space="PSUM"))

            identity = consts.tile([128, 128], f32)
            make_identity(nc, identity)

            bq_col = consts.tile([128, DK], f32)
            nc.sync.dma_start(out=bq_col, in_=col_ap(bq, DK))
            bk_col = consts.tile([128, DK], f32)
            nc.sync.dma_start(out=bk_col, in_=col_ap(bk, DK))
            bf1_col = consts.tile([128, PFK], f32)
            nc.sync.dma_start(out=bf1_col, in_=col_ap(bf1, PFK))

            eps_t = consts.tile([128, 1], f32)
            nc.vector.memset(eps_t, EPS)

            def layer_norm(r_row, g_bc, b_bc, out_tile, tmp_pool):
                """r_row [128, D] f32 -> out_tile [128, D] f32 (may alias r_row)."""
                stats = tmp_pool.tile([128, 2, 6], f32, tag="ln_stats")
                rr = r_row.rearrange("p (a f) -> p a f", a=2)
                for a in range(2):
                    nc.vector.bn_stats(out=stats[:, a, :], in_=rr[:, a, :])
                mv = tmp_pool.tile([128, 2], f32, tag="ln_mv")
                nc.vector.bn_aggr(out=mv, in_=stats)
                rstd = tmp_pool.tile([128, 1], f32, tag="ln_rstd")
                nc.scalar.activation(
                    out=rstd, in_=mv[:, 1:2], func=AF.Sqrt, bias=eps_t, scale=1.0
                )
                nc.vector.reciprocal(out=rstd, in_=rstd)
                nc.vector.tensor_scalar(
                    out=out_tile,
                    in0=r_row,
                    scalar1=mv[:, 0:1],
                    scalar2=rstd,
                    op0=ALU.subtract,
                    op1=ALU.mult,
                )
                nc.vector.tensor_mul(out=out_tile, in0=out_tile, in1=g_bc)
                nc.vector.tensor_add(out=out_tile, in0=out_tile, in1=b_bc)

            # ============ Phase 0 + attention ============
            with contextlib.ExitStack() as attn_ctx:
                srctp = attn_ctx.enter_context(tc.tile_pool(name="srctp", bufs=1))
                srcT = srctp.tile([128, DK, S2], f32r)  # 8MB

                # -- transpose src into srcT: seq blocks of 512 --
                with tc.tile_pool(name="ph0", bufs=2) as ph0:
                    for blk in range(4):  # 4 blocks of 512 seq positions
                        half = src_q if blk < 2 else src_o
                        base = blk * 512
                        sts = []
                        for j in range(4):
                            st = ph0.tile([128, D], f32, tag="src_ld", bufs=6)
                            row0 = (blk % 2) * 512 + j * 128
                            nc.sync.dma_start(
                                out=st, in_=half[row0 : row0 + 128, :]
                            )
                            sts.append(st)
                        for k in range(DK):
                            ps = psum.tile([128, 512], f32, tag="mm", bufs=3)
                            for j in range(4):
                                nc.tensor.transpose(
                                    ps[:, j * 128 : (j + 1) * 128],
                                    sts[j][:, k * 128 : (k + 1) * 128],
                                    identity,
                                )
                            nc.vector.tensor_copy(
                                out=srcT[:, k, base : base + 512], in_=ps
                            )

                acts = attn_ctx.enter_context(tc.tile_pool(name="acts", bufs=1))
                bv_bc = acts.tile([128, D], f32)
                nc.gpsimd.dma_start(out=bv_bc, in_=bc_ap(bv, D))

                grp = attn_ctx.enter_context(tc.tile_pool(name="grp", bufs=1))
                wst = attn_ctx.enter_context(tc.tile_pool(name="wst", bufs=1))
                expp = attn_ctx.enter_context(tc.tile_pool(name="expp", bufs=2))
                nrm = attn_ctx.enter_context(tc.tile_pool(name="nrm", bufs=2))

                for g in range(NG):
                    gc0 = g * GW  # first output dim of this group

                    # -- weight slices for this group --
                    wk_s = wst.tile([128, DK, GW], f32r, tag="wk_s")
                    nc.sync.dma_start(
                        out=wk_s,
                        in_=Wk.rearrange("(a p) n -> p a n", p=128)[
                            :, :, gc0 : gc0 + GW
                        ],
                    )
                    wq_s = wst.tile([128, DK, GW], f32r, tag="wq_s")
                    nc.sync.dma_start(
                        out=wq_s,
                        in_=Wq.rearrange("(a p) n -> p a n", p=128)[
                            :, :, gc0 : gc0 + GW
                        ],
                    )
                    wv_s = wst.tile([128, DK, GW], f32r, tag="wv_s")
                    nc.sync.dma_start(
                        out=wv_s,
                        in_=Wv.rearrange("(a p) n -> p a n", p=128)[
                            :, :, gc0 : gc0 + GW
                        ],
                    )

                    # -- KT_g [GW, S2] f32 --
                    KT = grp.tile([128, GW // 128, S2], f32r, tag="KT")
                    for m in range(GW // 128):
                        for n in range(S2 // 512):
                            ps = psum.tile([128, 512], f32, tag="mm", bufs=3)
                            for k in range(DK):
                                nc.tensor.matmul(
                                    ps,
                                    wk_s[:, k, m * 128 : (m + 1) * 128],
                                    srcT[:, k, n * 512 : (n + 1) * 512],
                                    start=(k == 0),
                                    stop=(k == DK - 1),
                                )
                            nc.scalar.activation(
                                out=KT[:, m, n * 512 : (n + 1) * 512],
                                in_=ps,
                                func=AF.Identity,
                                bias=bk_col[:, (gc0 // 128) + m : (gc0 // 128) + m + 1],
                                scale=1.0,
                            )

                    # -- QT_g [GW, SQ] f32 --
                    QT = grp.tile([128, GW // 128, SQ], f32r, tag="QT")
                    for m in range(GW // 128):
                        for n in range(SQ // 512):
                            ps = psum.tile([128, 512], f32, tag="mm", bufs=3)
                            for k in range(DK):
                                nc.tensor.matmul(
                                    ps,
                                    wq_s[:, k, m * 128 : (m + 1) * 128],
                                    srcT[:, k, n * 512 : (n + 1) * 512],
                                    start=(k == 0),
                                    stop=(k == DK - 1),
                                )
                            nc.scalar.activation(
                                out=QT[:, m, n * 512 : (n + 1) * 512],
                                in_=ps,
                                func=AF.Identity,
                                bias=bq_col[:, (gc0 // 128) + m : (gc0 // 128) + m + 1],
                                scale=1.0,
                            )

                    # -- V_g [S2, HPG*(HD+1)] bf16, ones col per head --
                    V = grp.tile([128, S2 // 128, HPG, HD + 1], bf16, tag="V")
                    nc.vector.memset(V[:, :, :, HD : HD + 1], 1.0)
                    for ms in range(S2 // 128):
                        ps = psum.tile([128, GW], f32, tag="mm", bufs=3)
                        for k in range(DK):
                            nc.tensor.matmul(
                                ps,
                                srcT[:, k, ms * 128 : (ms + 1) * 128],
                                wv_s[:, k, :],
                                start=(k == 0),
                                stop=(k == DK - 1),
                            )
                        nc.vector.tensor_add(
                            out=V[:, ms, :, 0:HD],
                            in0=ps.rearrange("p (h d) -> p h d", h=HPG),
                            in1=bv_bc.rearrange("p (h d) -> p h d", h=H)[
                                :, HPG * g : HPG * (g + 1), :
                            ],
                        )

                    # -- attention per head / sq tile --
                    for hh in range(HPG):
                        m_h = hh // 2          # 128-chunk within group tiles
                        p0 = (hh % 2) * 64     # partition offset
                        for sq in range(SQ // 512):
                            expS = expp.tile([128, S2 // 128, 512], bf16, tag="expS")
                            for sk in range(S2 // 128):
                                ps = psum.tile([128, 512], f32, tag="sc", bufs=2)
                                nc.tensor.matmul(
                                    ps,
                                    KT[
                                            p0 : p0 + 64,
                                            m_h,
                                            sk * 128 : (sk + 1) * 128,
                                        ],
                                    QT[
                                            p0 : p0 + 64,
                                            m_h,
                                            sq * 512 : (sq + 1) * 512,
                                        ],
                                    start=True,
                                    stop=True,
                                )
                                nc.scalar.activation(
                                    out=expS[:, sk, :],
                                    in_=ps,
                                    func=AF.Exp,
                                    scale=SCALE,
                                )
                            pv = psum.tile([HD + 1, 512], f32, tag="pv", bufs=2)
                            for sk in range(S2 // 128):
                                nc.tensor.matmul(
                                    pv,
                                    V[:, sk, hh, :],
                                    expS[:, sk, :],
                                    start=(sk == 0),
                                    stop=(sk == S2 // 128 - 1),
                                )
                            recip = nrm.tile([1, 512], f32, tag="recip")
                            nc.vector.reciprocal(out=recip, in_=pv[HD : HD + 1, :])
                            rb = nrm.tile([64, 512], f32, tag="rb")
                            nc.gpsimd.partition_broadcast(rb, recip)
                            xt = nrm.tile([64, 512], f32r, tag="xt")
                            nc.vector.tensor_mul(out=xt, in0=pv[0:HD, :], in1=rb)
                            h_abs = g * HPG + hh
                            nc.sync.dma_start(
                                out=xT_dram[
                                    h_abs * HD : (h_abs + 1) * HD,
                                    sq * 512 : (sq + 1) * 512,
                                ],
                                in_=xt,
                            )

            # ============ out-projection + residual + LN1 ============
            src1T_pool = ctx.enter_context(tc.tile_pool(name="src1T", bufs=1))
            src1T = src1T_pool.tile([128, DK, SQ], f32r)

            with contextlib.ExitStack() as octx:
                opool = octx.enter_context(tc.tile_pool(name="oproj", bufs=1))
                otmp = octx.enter_context(tc.tile_pool(name="otmp", bufs=3))

                bo_bc = opool.tile([128, D], f32)
                nc.gpsimd.dma_start(out=bo_bc, in_=bc_ap(bo, D))
                g1_bc = opool.tile([128, D], f32)
                nc.gpsimd.dma_start(out=g1_bc, in_=bc_ap(g1, D))
                b1_bc = opool.tile([128, D], f32)
                nc.gpsimd.dma_start(out=b1_bc, in_=bc_ap(b1, D))

                wo_s = opool.tile([128, DK, D], f32r)
                nc.sync.dma_start(
                    out=wo_s, in_=Wo.rearrange("(a p) n -> p a n", p=128)
                )
                xts = opool.tile([128, DK, SQ], f32r)
                nc.sync.dma_start(
                    out=xts, in_=xT_dram.rearrange("(a p) n -> p a n", p=128)
                )

                for m in range(SQ // 128):
                    r_row = otmp.tile([128, D], f32, tag="r_row")
                    for n in range(2):
                        ps = psum.tile([128, 512], f32, tag="mm", bufs=3)
                        for k in range(DK):
                            nc.tensor.matmul(
                                ps,
                                xts[:, k, m * 128 : (m + 1) * 128],
                                wo_s[:, k, n * 512 : (n + 1) * 512],
                                start=(k == 0),
                                stop=(k == DK - 1),
                            )
                        sq_t = otmp.tile([128, 512], f32, tag="sq_ld", bufs=4)
                        nc.sync.dma_start(
                            out=sq_t,
                            in_=src_q[
                                m * 128 : (m + 1) * 128, n * 512 : (n + 1) * 512
                            ],
                        )
                        nc.vector.tensor_add(
                            out=r_row[:, n * 512 : (n + 1) * 512], in0=ps, in1=sq_t
                        )
                    nc.vector.tensor_add(out=r_row, in0=r_row, in1=bo_bc)
                    s1 = otmp.tile([128, D], f32, tag="s1")
                    layer_norm(r_row, g1_bc, b1_bc, s1, otmp)
                    nc.sync.dma_start(
                        out=src1_dram[m * 128 : (m + 1) * 128, :], in_=s1
                    )
                    for k in range(DK):
                        ps = psum.tile([128, 128], f32, tag="mm", bufs=3)
                        nc.tensor.transpose(
                            ps, s1[:, k * 128 : (k + 1) * 128], identity
                        )
                        nc.scalar.copy(
                            out=src1T[:, k, m * 128 : (m + 1) * 128], in_=ps
                        )

            # ============ FFN ============
            with contextlib.ExitStack() as fctx:
                hpool = fctx.enter_context(tc.tile_pool(name="hpool", bufs=1))
                hT = hpool.tile([128, PFK, SQ], bf16)  # 8MB
                r2p = fctx.enter_context(tc.tile_pool(name="r2p", bufs=1))
                r2 = r2p.tile([128, SQ // 128, D], f32)  # 4MB
                w2p = fctx.enter_context(tc.tile_pool(name="w2p", bufs=2))
                ftmp = fctx.enter_context(tc.tile_pool(name="ftmp", bufs=3))

                bf2_bc = r2p.tile([128, D], f32)
                nc.gpsimd.dma_start(out=bf2_bc, in_=bc_ap(bf2, D))
                g2_bc = r2p.tile([128, D], f32)
                nc.gpsimd.dma_start(out=g2_bc, in_=bc_ap(g2, D))
                b2_bc = r2p.tile([128, D], f32)
                nc.gpsimd.dma_start(out=b2_bc, in_=bc_ap(b2, D))

                # FFN1: hT[pf, q] = relu(W1^T src1T + bf1)
                for mp in range(PFK):
                    w1_s = ftmp.tile([128, DK, 128], f32r, tag="w1_s", bufs=2)
                    nc.sync.dma_start(
                        out=w1_s,
                        in_=W1.rearrange("(a p) n -> p a n", p=128)[
                            :, :, mp * 128 : (mp + 1) * 128
                        ],
                    )
                    for sq in range(SQ // 512):
                        ps = psum.tile([128, 512], f32, tag="mm", bufs=3)
                        for k in range(DK):
                            nc.tensor.matmul(
                                ps,
                                w1_s[:, k, :],
                                src1T[:, k, sq * 512 : (sq + 1) * 512],
                                start=(k == 0),
                                stop=(k == DK - 1),
                            )
                        nc.scalar.activation(
                            out=hT[:, mp, sq * 512 : (sq + 1) * 512],
                            in_=ps,
                            func=AF.Relu,
                            bias=bf1_col[:, mp : mp + 1],
                            scale=1.0,
                        )

                # FFN2 in D-quarters of 256, W2 cast to bf16 on the fly
                for nq in range(4):
                    w2bf = w2p.tile([128, PFK, 256], bf16, tag="w2bf")
                    for k in range(PFK):
                        w2_ld = ftmp.tile([128, 256], f32, tag="w2_ld", bufs=3)
                        nc.sync.dma_start(
                            out=w2_ld,
                            in_=W2[
                                k * 128 : (k + 1) * 128, nq * 256 : (nq + 1) * 256
                            ],
                        )
                        nc.vector.tensor_copy(out=w2bf[:, k, :], in_=w2_ld)
                    for m in range(SQ // 128):
                        ps = psum.tile([128, 256], f32, tag="mm", bufs=3)
                        for k in range(PFK):
                            nc.tensor.matmul(
                                ps,
                                hT[:, k, m * 128 : (m + 1) * 128],
                                w2bf[:, k, :],
                                start=(k == 0),
                                stop=(k == PFK - 1),
                            )
                        nc.scalar.copy(
                            out=r2[:, m, nq * 256 : (nq + 1) * 256], in_=ps
                        )

                # residual + LN2 + store
                for m in range(SQ // 128):
                    s1_t = ftmp.tile([128, D], f32, tag="s1_ld", bufs=2)
                    nc.sync.dma_start(
                        out=s1_t, in_=src1_dram[m * 128 : (m + 1) * 128, :]
                    )
                    rr = ftmp.tile([128, D], f32, tag="rr", bufs=2)
                    nc.vector.tensor_add(out=rr, in0=r2[:, m, :], in1=s1_t)
                    nc.vector.tensor_add(out=rr, in0=rr, in1=bf2_bc)
                    layer_norm(rr, g2_bc, b2_bc, rr, ftmp)
                    nc.sync.dma_start(out=out[m * 128 : (m + 1) * 128, :], in_=rr)

    nc.compile()
    return nc


def kernel(**inputs):
    from concourse.bass_utils import run_bass_kernel_spmd

    if "nc" not in _CACHE:
        _CACHE["nc"] = _build()
    nc = _CACHE["nc"]

    ins = {k: np.asarray(v, dtype=np.float32) for k, v in inputs.items()}
    src = ins["src"]
    weight_names = [
        "Wq", "Wk", "Wv", "Wo", "W1", "W2",
        "bq", "bk", "bv", "bo", "bf1", "bf2", "g1", "b1", "g2", "b2",
    ]
    weights = {n: np.ascontiguousarray(ins[n]) for n in weight_names}

    in_maps = []
    for c in range(N_CORES):
        b, h = divmod(c, 2)
        m = dict(weights)
        m["src_q"] = np.ascontiguousarray(src[b, h * SQ : (h + 1) * SQ])
        m["src_o"] = np.ascontiguousarray(src[b, (1 - h) * SQ : (2 - h) * SQ])
        in_maps.append(m)

    res = run_bass_kernel_spmd(nc, in_maps, list(range(N_CORES)))

    out = np.empty((4, S2, D), dtype=np.float32)
    for c in range(N_CORES):
        b, h = divmod(c, 2)
        out[b, h * SQ : (h + 1) * SQ] = res.results[c]["out"]
    return out


# revision 13
# speedup vs baseline: 1.1236x; 1.1236x over previous
"""Trainium2 Bass kernel for a transformer encoder layer.

B=4, S=2048, D=1024, H=16 heads (HD=64), PF=4096, fp32 I/O.

Sharding: 8 cores, core c handles batch c//2, query seq-half c%2 (1024
tokens). Each core computes K/V over its batch's full 2048-token sequence
(duplicated within the pair; ~12% extra flops) so no collectives are needed.

Dataflow (per core) keeps attention transposed so no P-matrix transpose is
ever required:
  srcT = src^T (TensorE transposes)
  QT = Wq^T srcT_q, KT = Wk^T srcT (f32r), V = srcT^T Wv (bf16, with a
      per-head ones column appended so PV^T also yields softmax denominators)
  S^T = K Q^T (f32r, contraction=HD), expS = exp(S^T/8) (bf16)
  x^T = V_aug^T expS (bf16, contraction=Sk) -> divide by denom row
  attn_out = (x^T)^T Wo, + src + bo, LN1 -> src1 (+ src1^T)
  h^T = relu(W1^T src1^T + bf1) (bf16), ff = (h^T)^T W2 + bf2
  out = LN2(src1 + ff)
"""

import numpy as np

D = 1024
S2 = 2048
SQ = 1024
PF = 4096
H = 16
HD = 64
DK = D // 128          # 8 contraction chunks over D
PFK = PF // 128        # 32 contraction chunks over PF
NG = 4                 # head groups
HPG = H // NG          # 4 heads per group
GW = HPG * HD          # 256 output dims per group
GM = GW // 128         # 128-chunks per group
SCALE = 1.0 / 8.0      # 1/sqrt(HD)
EPS = 1e-5
N_CORES = 8

_CACHE = {}


def _build():
    import concourse.bass as bass
    import concourse.mybir as mybir
    import concourse.tile as tile
    from concourse import bacc
    from concourse.masks import make_identity

    f32 = mybir.dt.float32
    f32r = mybir.dt.float32r
    bf16 = mybir.dt.bfloat16
    AF = mybir.ActivationFunctionType
    ALU = mybir.AluOpType

    nc = bacc.Bacc("TRN2", target_bir_lowering=False, debug=False, num_devices=N_CORES)

    def din(name, shape, dt=f32):
        return nc.dram_tensor(name, shape, dt, kind="ExternalInput")

    src_q = din("src_q", [SQ, D])     # this core's query tokens
    src_o = din("src_o", [SQ, D])     # the other half of the sequence
    Wq = din("Wq", [D, D], f32r)
    Wk = din("Wk", [D, D], f32r)
    Wv = din("Wv", [D, D], f32r)
    Wo = din("Wo", [D, D], f32r)
    W1 = din("W1", [D, PF], f32r)
    W2 = din("W2", [PF, D])
    bq = din("bq", [D])
    bk = din("bk", [D])
    bv = din("bv", [D])
    bo = din("bo", [D])
    bf1 = din("bf1", [PF])
    bf2 = din("bf2", [D])
    g1 = din("g1", [D])
    b1 = din("b1", [D])
    g2 = din("g2", [D])
    b2 = din("b2", [D])
    out = nc.dram_tensor("out", [SQ, D], f32, kind="ExternalOutput")

    xT_dram = nc.dram_tensor("xT_scratch", [D, SQ], f32r)
    src1_dram = nc.dram_tensor("src1_scratch", [SQ, D], f32)

    def bc_ap(vec, n):
        # [n] dram vector broadcast to [128, n]
        return bass.AP(tensor=vec, offset=0, ap=[[0, 128], [1, n]])

    def col_ap(vec, m):
        # [m*128] dram vector viewed as [128, m] columns
        return bass.AP(tensor=vec, offset=0, ap=[[1, 128], [128, m]])

    with tile.TileContext(nc) as tc:
        import contextlib

        with contextlib.ExitStack() as ctx:
            consts = ctx.enter_context(tc.tile_pool(name="consts", bufs=1))
            psum = ctx.enter_context(tc.tile_pool(name="psum", bufs=1, space="PSUM"))

            identity = consts.tile([128, 128], f32)
            make_identity(nc, identity)

            bq_col = consts.tile([128, DK], f32)
            nc.sync.dma_start(out=bq_col, in_=col_ap(bq, DK))
            bk_col = consts.tile([128, DK], f32)
            nc.sync.dma_start(out=bk_col, in_=col_ap(bk, DK))
            bf1_col = consts.tile([128, PFK], f32)
            nc.sync.dma_start(out=bf1_col, in_=col_ap(bf1, PFK))

            eps_t = consts.tile([128, 1], f32)
            nc.vector.memset(eps_t, EPS)

            def layer_norm(r_row, g_bc, b_bc, out_tile, tmp_pool):
                """r_row [128, D] f32 -> out_tile (may alias r_row)."""
                stats = tmp_pool.tile([128, 2, 6], f32, tag="ln_stats")
                rr = r_row.rearrange("p (a f) -> p a f", a=2)
                for a in range(2):
                    nc.vector.bn_stats(out=stats[:, a, :], in_=rr[:, a, :])
                mv = tmp_pool.tile([128, 2], f32, tag="ln_mv")
                nc.vector.bn_aggr(out=mv, in_=stats)
                rstd = tmp_pool.tile([128, 1], f32, tag="ln_rstd")
                nc.scalar.activation(
                    out=rstd, in_=mv[:, 1:2], func=AF.Sqrt, bias=eps_t, scale=1.0
                )
                nc.vector.reciprocal(out=rstd, in_=rstd)
                nc.vector.tensor_scalar(
                    out=out_tile,
                    in0=r_row,
                    scalar1=mv[:, 0:1],
                    scalar2=rstd,
                    op0=ALU.subtract,
                    op1=ALU.mult,
                )
                nc.vector.tensor_mul(out=out_tile, in0=out_tile, in1=g_bc)
                nc.vector.tensor_add(out=out_tile, in0=out_tile, in1=b_bc)

            # ============ Phase 0 + attention ============
            with contextlib.ExitStack() as attn_ctx:
                srctp = attn_ctx.enter_context(tc.tile_pool(name="srctp", bufs=1))
                srcT = srctp.tile([128, DK, S2], f32r)  # 8MB

                # -- transpose src into srcT: seq blocks of 512 --
                with tc.tile_pool(name="ph0", bufs=2) as ph0:
                    for blk in range(4):
                        half = src_q if blk < 2 else src_o
                        base = blk * 512
                        sts = []
                        for j in range(4):
                            st = ph0.tile([128, D], f32, tag="src_ld", bufs=6)
                            row0 = (blk % 2) * 512 + j * 128
                            nc.sync.dma_start(out=st, in_=half[row0 : row0 + 128, :])
                            sts.append(st)
                        for k in range(DK):
                            ps = psum.tile([128, 512], f32, tag="mm", bufs=4)
                            for j in range(4):
                                nc.tensor.transpose(
                                    ps[:, j * 128 : (j + 1) * 128],
                                    sts[j][:, k * 128 : (k + 1) * 128],
                                    identity,
                                )
                            nc.vector.tensor_copy(
                                out=srcT[:, k, base : base + 512], in_=ps
                            )

                acts = attn_ctx.enter_context(tc.tile_pool(name="acts", bufs=1))
                bv_bc = acts.tile([128, D], f32)
                nc.gpsimd.dma_start(out=bv_bc, in_=bc_ap(bv, D))

                grp = attn_ctx.enter_context(tc.tile_pool(name="grp", bufs=1))
                wst = attn_ctx.enter_context(tc.tile_pool(name="wst", bufs=1))
                expp = attn_ctx.enter_context(tc.tile_pool(name="expp", bufs=2))
                nrm = attn_ctx.enter_context(tc.tile_pool(name="nrm", bufs=2))

                for g in range(NG):
                    gc0 = g * GW

                    wk_s = wst.tile([128, DK, GW], f32r, tag="wk_s")
                    nc.sync.dma_start(
                        out=wk_s,
                        in_=Wk.rearrange("(a p) n -> p a n", p=128)[:, :, gc0 : gc0 + GW],
                    )
                    wq_s = wst.tile([128, DK, GW], f32r, tag="wq_s")
                    nc.sync.dma_start(
                        out=wq_s,
                        in_=Wq.rearrange("(a p) n -> p a n", p=128)[:, :, gc0 : gc0 + GW],
                    )
                    wv_s = wst.tile([128, DK, GW], f32r, tag="wv_s")
                    nc.sync.dma_start(
                        out=wv_s,
                        in_=Wv.rearrange("(a p) n -> p a n", p=128)[:, :, gc0 : gc0 + GW],
                    )

                    # -- KT_g [GW, S2]: weight stationary over 4 n-tiles --
                    KT = grp.tile([128, GM, S2], f32r, tag="KT")
                    for m in range(GM):
                        pss = []
                        for n in range(4):
                            ps = psum.tile([128, 512], f32, tag="mm", bufs=4)
                            pss.append(ps)
                        for k in range(DK):
                            for n in range(4):
                                nc.tensor.matmul(
                                    pss[n],
                                    wk_s[:, k, m * 128 : (m + 1) * 128],
                                    srcT[:, k, n * 512 : (n + 1) * 512],
                                    start=(k == 0),
                                    stop=(k == DK - 1),
                                )
                        for n in range(4):
                            nc.vector.tensor_scalar_add(
                                out=KT[:, m, n * 512 : (n + 1) * 512],
                                in0=pss[n],
                                scalar1=bk_col[:, (gc0 // 128) + m : (gc0 // 128) + m + 1],
                            )

                    # -- QT_g [GW, SQ]: weight stationary over 2 n-tiles --
                    QT = grp.tile([128, GM, SQ], f32r, tag="QT")
                    for m in range(GM):
                        pss = []
                        for n in range(2):
                            ps = psum.tile([128, 512], f32, tag="mm", bufs=4)
                            pss.append(ps)
                        for k in range(DK):
                            for n in range(2):
                                nc.tensor.matmul(
                                    pss[n],
                                    wq_s[:, k, m * 128 : (m + 1) * 128],
                                    srcT[:, k, n * 512 : (n + 1) * 512],
                                    start=(k == 0),
                                    stop=(k == DK - 1),
                                )
                        for n in range(2):
                            nc.vector.tensor_scalar_add(
                                out=QT[:, m, n * 512 : (n + 1) * 512],
                                in0=pss[n],
                                scalar1=bq_col[:, (gc0 // 128) + m : (gc0 // 128) + m + 1],
                            )

                    # -- V_g [S2, HPG*(HD+1)] bf16, ones col per head --
                    V = grp.tile([128, S2 // 128, HPG, HD + 1], bf16, tag="V")
                    nc.vector.memset(V[:, :, :, HD : HD + 1], 1.0)
                    for ms in range(S2 // 128):
                        ps = psum.tile([128, GW], f32, tag="mm", bufs=4)
                        for k in range(DK):
                            nc.tensor.matmul(
                                ps,
                                srcT[:, k, ms * 128 : (ms + 1) * 128],
                                wv_s[:, k, :],
                                start=(k == 0),
                                stop=(k == DK - 1),
                            )
                        nc.vector.tensor_add(
                            out=V[:, ms, :, 0:HD],
                            in0=ps.rearrange("p (h d) -> p h d", h=HPG),
                            in1=bv_bc.rearrange("p (h d) -> p h d", h=H)[
                                :, HPG * g : HPG * (g + 1), :
                            ],
                        )

                    # -- attention per head / sq tile --
                    for hh in range(HPG):
                        m_h = hh // 2
                        p0 = (hh % 2) * 64
                        for sq in range(SQ // 512):
                            expS = expp.tile([128, S2 // 128, 512], bf16, tag="expS")
                            for sk in range(S2 // 128):
                                ps = psum.tile([128, 512], f32, tag="sc", bufs=2)
                                nc.tensor.matmul(
                                    ps,
                                    KT[p0 : p0 + 64, m_h, sk * 128 : (sk + 1) * 128],
                                    QT[p0 : p0 + 64, m_h, sq * 512 : (sq + 1) * 512],
                                    start=True,
                                    stop=True,
                                )
                                nc.scalar.activation(
                                    out=expS[:, sk, :], in_=ps, func=AF.Exp, scale=SCALE
                                )
                            pv = psum.tile([HD + 1, 512], f32, tag="pv", bufs=2)
                            for sk in range(S2 // 128):
                                nc.tensor.matmul(
                                    pv,
                                    V[:, sk, hh, :],
                                    expS[:, sk, :],
                                    start=(sk == 0),
                                    stop=(sk == S2 // 128 - 1),
                                )
                            den = nrm.tile([1, 512], f32, tag="den")
                            nc.vector.tensor_copy(out=den, in_=pv[HD : HD + 1, :])
                            den_bc = nrm.tile([64, 512], f32, tag="den_bc")
                            nc.gpsimd.partition_broadcast(den_bc, den)
                            nc.vector.reciprocal(out=den_bc, in_=den_bc)
                            xt = nrm.tile([64, 512], f32r, tag="xt")
                            nc.vector.tensor_mul(out=xt, in0=pv[0:HD, :], in1=den_bc)
                            h_abs = g * HPG + hh
                            nc.sync.dma_start(
                                out=xT_dram[
                                    h_abs * HD : (h_abs + 1) * HD,
                                    sq * 512 : (sq + 1) * 512,
                                ],
                                in_=xt,
                            )

            # ============ out-projection + residual + LN1, then FFN ============
            with contextlib.ExitStack() as fo_ctx:
                src1T_pool = fo_ctx.enter_context(tc.tile_pool(name="src1T", bufs=1))
                src1T = src1T_pool.tile([128, DK, SQ], f32r)

                with contextlib.ExitStack() as octx:
                    opool = octx.enter_context(tc.tile_pool(name="oproj", bufs=1))
                    otmp = octx.enter_context(tc.tile_pool(name="otmp", bufs=2))

                    bo_bc = opool.tile([128, D], f32)
                    nc.gpsimd.dma_start(out=bo_bc, in_=bc_ap(bo, D))
                    g1_bc = opool.tile([128, D], f32)
                    nc.gpsimd.dma_start(out=g1_bc, in_=bc_ap(g1, D))
                    b1_bc = opool.tile([128, D], f32)
                    nc.gpsimd.dma_start(out=b1_bc, in_=bc_ap(b1, D))

                    # chunked loads so matmuls start after the first chunk
                    wo_s = opool.tile([128, DK, D], f32r)
                    xts = opool.tile([128, DK, SQ], f32r)
                    for k in range(DK):
                        nc.sync.dma_start(
                            out=xts[:, k, :],
                            in_=xT_dram[k * 128 : (k + 1) * 128, :],
                        )
                        nc.sync.dma_start(
                            out=wo_s[:, k, :],
                            in_=Wo[k * 128 : (k + 1) * 128, :],
                        )

                    r_sb = opool.tile([128, SQ // 128, D], f32)

                    # pass A: all matmuls + residual adds
                    for m in range(SQ // 128):
                        pss = []
                        for n in range(2):
                            ps = psum.tile([128, 512], f32, tag="mm", bufs=4)
                            pss.append(ps)
                        for k in range(DK):
                            for n in range(2):
                                nc.tensor.matmul(
                                    pss[n],
                                    xts[:, k, m * 128 : (m + 1) * 128],
                                    wo_s[:, k, n * 512 : (n + 1) * 512],
                                    start=(k == 0),
                                    stop=(k == DK - 1),
                                )
                        sq_t = otmp.tile([128, D], f32, tag="sq_ld", bufs=3)
                        nc.sync.dma_start(out=sq_t, in_=src_q[m * 128 : (m + 1) * 128, :])
                        for n in range(2):
                            nc.vector.tensor_add(
                                out=r_sb[:, m, n * 512 : (n + 1) * 512],
                                in0=pss[n],
                                in1=sq_t[:, n * 512 : (n + 1) * 512],
                            )

                    # pass B: LN1 + transposes into src1T (+ src1 to DRAM)
                    for m in range(SQ // 128):
                        rrow = r_sb[:, m, :]
                        nc.vector.tensor_add(out=rrow, in0=rrow, in1=bo_bc)
                        s1 = otmp.tile([128, D], f32, tag="s1", bufs=3)
                        layer_norm(rrow, g1_bc, b1_bc, s1, otmp)
                        nc.sync.dma_start(
                            out=src1_dram[m * 128 : (m + 1) * 128, :], in_=s1
                        )
                        for kk in range(2):
                            ps = psum.tile([128, 512], f32, tag="mm", bufs=4)
                            for j in range(4):
                                k = kk * 4 + j
                                nc.tensor.transpose(
                                    ps[:, j * 128 : (j + 1) * 128],
                                    s1[:, k * 128 : (k + 1) * 128],
                                    identity,
                                )
                            for j in range(4):
                                k = kk * 4 + j
                                nc.vector.tensor_copy(
                                    out=src1T[:, k, m * 128 : (m + 1) * 128],
                                    in_=ps[:, j * 128 : (j + 1) * 128],
                                )

                # ============ FFN ============
                with contextlib.ExitStack() as fctx:
                    hpool = fctx.enter_context(tc.tile_pool(name="hpool", bufs=1))
                    hT = hpool.tile([128, PFK, SQ], bf16)       # 8MB
                    w2p = fctx.enter_context(tc.tile_pool(name="w2p", bufs=1))
                    w2bf = w2p.tile([128, PFK, D], bf16)        # 8MB
                    fcts = fctx.enter_context(tc.tile_pool(name="fcts", bufs=1))
                    ftmp = fctx.enter_context(tc.tile_pool(name="ftmp", bufs=2))

                    bf2_bc = fcts.tile([128, D], f32)
                    nc.gpsimd.dma_start(out=bf2_bc, in_=bc_ap(bf2, D))
                    g2_bc = fcts.tile([128, D], f32)
                    nc.gpsimd.dma_start(out=g2_bc, in_=bc_ap(g2, D))
                    b2_bc = fcts.tile([128, D], f32)
                    nc.gpsimd.dma_start(out=b2_bc, in_=bc_ap(b2, D))

                    # W2 load + cast to bf16 (overlaps FFN1)
                    for k in range(PFK):
                        w2_ld = ftmp.tile([128, D], f32, tag="w2_ld", bufs=2)
                        nc.sync.dma_start(out=w2_ld, in_=W2[k * 128 : (k + 1) * 128, :])
                        nc.vector.tensor_copy(out=w2bf[:, k, :], in_=w2_ld)

                    # FFN1: hT[pf, q] = relu(W1^T src1T + bf1), weight stationary
                    for mp in range(PFK):
                        w1_s = ftmp.tile([128, DK, 128], f32r, tag="w1_s", bufs=2)
                        nc.sync.dma_start(
                            out=w1_s,
                            in_=W1.rearrange("(a p) n -> p a n", p=128)[
                                :, :, mp * 128 : (mp + 1) * 128
                            ],
                        )
                        pss = []
                        for sqh in range(2):
                            ps = psum.tile([128, 512], f32, tag="mm", bufs=4)
                            pss.append(ps)
                        for k in range(DK):
                            for sqh in range(2):
                                nc.tensor.matmul(
                                    pss[sqh],
                                    w1_s[:, k, :],
                                    src1T[:, k, sqh * 512 : (sqh + 1) * 512],
                                    start=(k == 0),
                                    stop=(k == DK - 1),
                                )
                        for sqh in range(2):
                            nc.vector.tensor_scalar(
                                out=hT[:, mp, sqh * 512 : (sqh + 1) * 512],
                                in0=pss[sqh],
                                scalar1=bf1_col[:, mp : mp + 1],
                                scalar2=0.0,
                                op0=ALU.add,
                                op1=ALU.max,
                            )

                    # FFN2: per-m row, weight(hT) stationary over 2 n-halves
                    for m in range(SQ // 128):
                        pss = []
                        for n in range(2):
                            ps = psum.tile([128, 512], f32, tag="mm", bufs=4)
                            pss.append(ps)
                        for k in range(PFK):
                            for n in range(2):
                                nc.tensor.matmul(
                                    pss[n],
                                    hT[:, k, m * 128 : (m + 1) * 128],
                                    w2bf[:, k, n * 512 : (n + 1) * 512],
                                    start=(k == 0),
                                    stop=(k == PFK - 1),
                                )
                        s1_t = ftmp.tile([128, D], f32, tag="s1_ld", bufs=2)
                        nc.sync.dma_start(
                            out=s1_t, in_=src1_dram[m * 128 : (m + 1) * 128, :]
                        )
                        rr = ftmp.tile([128, D], f32, tag="rr", bufs=2)
                        for n in range(2):
                            nc.vector.tensor_add(
                                out=rr[:, n * 512 : (n + 1) * 512],
                                in0=pss[n],
                                in1=s1_t[:, n * 512 : (n + 1) * 512],
                            )
                        nc.vector.tensor_add(out=rr, in0=rr, in1=bf2_bc)
                        layer_norm(rr, g2_bc, b2_bc, rr, ftmp)
                        nc.sync.dma_start(out=out[m * 128 : (m + 1) * 128, :], in_=rr)

    nc.compile()
    return nc


def kernel(**inputs):
    from concourse.bass_utils import run_bass_kernel_spmd

    if "nc" not in _CACHE:
        _CACHE["nc"] = _build()
    nc = _CACHE["nc"]

    ins = {k: np.asarray(v, dtype=np.float32) for k, v in inputs.items()}
    src = ins["src"]
    weight_names = [
        "Wq", "Wk", "Wv", "Wo", "W1", "W2",
        "bq", "bk", "bv", "bo", "bf1", "bf2", "g1", "b1", "g2", "b2",
    ]
    weights = {n: np.ascontiguousarray(ins[n]) for n in weight_names}

    in_maps = []
    for c in range(N_CORES):
        b, h = divmod(c, 2)
        m = dict(weights)
        m["src_q"] = np.ascontiguousarray(src[b, h * SQ : (h + 1) * SQ])
        m["src_o"] = np.ascontiguousarray(src[b, (1 - h) * SQ : (2 - h) * SQ])
        in_maps.append(m)

    res = run_bass_kernel_spmd(nc, in_maps, list(range(N_CORES)))

    out = np.empty((4, S2, D), dtype=np.float32)
    for c in range(N_CORES):
        b, h = divmod(c, 2)
        out[b, h * SQ : (h + 1) * SQ] = res.results[c]["out"]
    return out


# revision 14
# speedup vs baseline: 1.1607x; 1.0330x over previous
"""Trainium2 Bass kernel for a transformer encoder layer.

B=4, S=2048, D=1024, H=16 heads (HD=64), PF=4096, fp32 I/O.

Sharding: 8 cores, core c handles batch c//2, query seq-half c%2 (1024
tokens). Each core computes K/V over its batch's full 2048-token sequence
(duplicated within the pair; ~12% extra flops) so no collectives are needed.

Dataflow (per core) keeps attention transposed so no P-matrix transpose is
ever required:
  srcT = src^T (TensorE transposes)
  QT = Wq^T srcT_q, KT = Wk^T srcT (f32r), V = srcT^T Wv (bf16, with a
      per-head ones column appended so PV^T also yields softmax denominators)
  S^T = K Q^T (f32r, contraction=HD), expS = exp(S^T/8) (bf16)
  x^T = V_aug^T expS (bf16, contraction=Sk) -> divide by denom row
  attn_out = (x^T)^T Wo, + src + bo, LN1 -> src1 (+ src1^T)
  h^T = relu(W1^T src1^T + bf1) (bf16), ff = (h^T)^T W2 + bf2
  out = LN2(src1 + ff)
"""

import numpy as np

D = 1024
S2 = 2048
SQ = 1024
PF = 4096
H = 16
HD = 64
DK = D // 128          # 8 contraction chunks over D
PFK = PF // 128        # 32 contraction chunks over PF
NG = 4                 # head groups
HPG = H // NG          # 4 heads per group
GW = HPG * HD          # 256 output dims per group
GM = GW // 128         # 128-chunks per group
SCALE = 1.0 / 8.0      # 1/sqrt(HD)
EPS = 1e-5
N_CORES = 8

_CACHE = {}


def _build():
    import concourse.bass as bass
    import concourse.mybir as mybir
    import concourse.tile as tile
    from concourse import bacc
    from concourse.masks import make_identity

    f32 = mybir.dt.float32
    f32r = mybir.dt.float32r
    bf16 = mybir.dt.bfloat16
    AF = mybir.ActivationFunctionType
    ALU = mybir.AluOpType

    nc = bacc.Bacc("TRN2", target_bir_lowering=False, debug=False, num_devices=N_CORES)

    def din(name, shape, dt=f32):
        return nc.dram_tensor(name, shape, dt, kind="ExternalInput")

    src_q = din("src_q", [SQ, D])     # this core's query tokens
    src_o = din("src_o", [SQ, D])     # the other half of the sequence
    Wq = din("Wq", [D, D], f32r)
    Wk = din("Wk", [D, D], f32r)
    Wv = din("Wv", [D, D], f32r)
    Wo = din("Wo", [D, D], f32r)
    W1 = din("W1", [D, PF], f32r)
    W2 = din("W2", [PF, D])
    bq = din("bq", [D])
    bk = din("bk", [D])
    bv = din("bv", [D])
    bo = din("bo", [D])
    bf1 = din("bf1", [PF])
    bf2 = din("bf2", [D])
    g1 = din("g1", [D])
    b1 = din("b1", [D])
    g2 = din("g2", [D])
    b2 = din("b2", [D])
    out = nc.dram_tensor("out", [SQ, D], f32, kind="ExternalOutput")

    xT_dram = nc.dram_tensor("xT_scratch", [D, SQ], f32r)
    src1_dram = nc.dram_tensor("src1_scratch", [SQ, D], f32)

    def bc_ap(vec, n):
        # [n] dram vector broadcast to [128, n]
        return bass.AP(tensor=vec, offset=0, ap=[[0, 128], [1, n]])

    def col_ap(vec, m):
        # [m*128] dram vector viewed as [128, m] columns
        return bass.AP(tensor=vec, offset=0, ap=[[1, 128], [128, m]])

    with tile.TileContext(nc) as tc:
        import contextlib

        with contextlib.ExitStack() as ctx:
            consts = ctx.enter_context(tc.tile_pool(name="consts", bufs=1))
            psum = ctx.enter_context(tc.tile_pool(name="psum", bufs=1, space="PSUM"))

            identity = consts.tile([128, 128], f32)
            make_identity(nc, identity)

            bq_col = consts.tile([128, DK], f32)
            nc.sync.dma_start(out=bq_col, in_=col_ap(bq, DK))
            bk_col = consts.tile([128, DK], f32)
            nc.sync.dma_start(out=bk_col, in_=col_ap(bk, DK))
            bf1_col = consts.tile([128, PFK], f32)
            nc.sync.dma_start(out=bf1_col, in_=col_ap(bf1, PFK))

            eps_t = consts.tile([128, 1], f32)
            nc.vector.memset(eps_t, EPS)

            def layer_norm(r_row, g_bc, b_bc, out_tile, tmp_pool):
                """r_row [128, D] f32 -> out_tile (may alias r_row)."""
                stats = tmp_pool.tile([128, 2, 6], f32, tag="ln_stats")
                rr = r_row.rearrange("p (a f) -> p a f", a=2)
                for a in range(2):
                    nc.vector.bn_stats(out=stats[:, a, :], in_=rr[:, a, :])
                mv = tmp_pool.tile([128, 2], f32, tag="ln_mv")
                nc.vector.bn_aggr(out=mv, in_=stats)
                rstd = tmp_pool.tile([128, 1], f32, tag="ln_rstd")
                nc.scalar.activation(
                    out=rstd, in_=mv[:, 1:2], func=AF.Sqrt, bias=eps_t, scale=1.0
                )
                nc.vector.reciprocal_approx_fast(out=rstd, in_=rstd)
                nc.vector.tensor_scalar(
                    out=out_tile,
                    in0=r_row,
                    scalar1=mv[:, 0:1],
                    scalar2=rstd,
                    op0=ALU.subtract,
                    op1=ALU.mult,
                )
                nc.vector.tensor_mul(out=out_tile, in0=out_tile, in1=g_bc)
                nc.vector.tensor_add(out=out_tile, in0=out_tile, in1=b_bc)

            # ============ Phase 0 + attention ============
            with contextlib.ExitStack() as attn_ctx:
                srctp = attn_ctx.enter_context(tc.tile_pool(name="srctp", bufs=1))
                srcT = srctp.tile([128, DK, S2], f32r)  # 8MB

                # -- transpose src into srcT: seq blocks of 512 --
                with tc.tile_pool(name="ph0", bufs=2) as ph0:
                    for blk in range(4):
                        half = src_q if blk < 2 else src_o
                        base = blk * 512
                        sts = []
                        for j in range(4):
                            st = ph0.tile([128, D], f32, tag="src_ld", bufs=6)
                            row0 = (blk % 2) * 512 + j * 128
                            nc.sync.dma_start(out=st, in_=half[row0 : row0 + 128, :])
                            sts.append(st)
                        for k in range(DK):
                            ps = psum.tile([128, 512], f32, tag="mm", bufs=4)
                            for j in range(4):
                                nc.tensor.transpose(
                                    ps[:, j * 128 : (j + 1) * 128],
                                    sts[j][:, k * 128 : (k + 1) * 128],
                                    identity,
                                )
                            nc.vector.tensor_copy(
                                out=srcT[:, k, base : base + 512], in_=ps
                            )

                acts = attn_ctx.enter_context(tc.tile_pool(name="acts", bufs=1))
                bv_bc = acts.tile([128, D], f32)
                nc.gpsimd.dma_start(out=bv_bc, in_=bc_ap(bv, D))

                grp = attn_ctx.enter_context(tc.tile_pool(name="grp", bufs=2))
                wst = attn_ctx.enter_context(tc.tile_pool(name="wst", bufs=1))
                expp = attn_ctx.enter_context(tc.tile_pool(name="expp", bufs=2))
                nrm = attn_ctx.enter_context(tc.tile_pool(name="nrm", bufs=2))

                for g in range(NG):
                    gc0 = g * GW

                    wk_s = wst.tile([128, DK, GW], f32r, tag="wk_s")
                    nc.sync.dma_start(
                        out=wk_s,
                        in_=Wk.rearrange("(a p) n -> p a n", p=128)[:, :, gc0 : gc0 + GW],
                    )
                    wq_s = wst.tile([128, DK, GW], f32r, tag="wq_s")
                    nc.sync.dma_start(
                        out=wq_s,
                        in_=Wq.rearrange("(a p) n -> p a n", p=128)[:, :, gc0 : gc0 + GW],
                    )
                    wv_s = wst.tile([128, DK, GW], f32r, tag="wv_s")
                    nc.sync.dma_start(
                        out=wv_s,
                        in_=Wv.rearrange("(a p) n -> p a n", p=128)[:, :, gc0 : gc0 + GW],
                    )

                    # -- KT_g [GW, S2]: weight stationary over 4 n-tiles --
                    KT = grp.tile([128, GM, S2], f32r, tag="KT")
                    for m in range(GM):
                        pss = []
                        for n in range(4):
                            ps = psum.tile([128, 512], f32, tag="mm", bufs=4)
                            pss.append(ps)
                        for k in range(DK):
                            for n in range(4):
                                nc.tensor.matmul(
                                    pss[n],
                                    wk_s[:, k, m * 128 : (m + 1) * 128],
                                    srcT[:, k, n * 512 : (n + 1) * 512],
                                    start=(k == 0),
                                    stop=(k == DK - 1),
                                )
                        for n in range(4):
                            nc.vector.tensor_scalar_add(
                                out=KT[:, m, n * 512 : (n + 1) * 512],
                                in0=pss[n],
                                scalar1=bk_col[:, (gc0 // 128) + m : (gc0 // 128) + m + 1],
                            )

                    # -- QT_g [GW, SQ]: weight stationary over 2 n-tiles --
                    QT = grp.tile([128, GM, SQ], f32r, tag="QT")
                    for m in range(GM):
                        pss = []
                        for n in range(2):
                            ps = psum.tile([128, 512], f32, tag="mm", bufs=4)
                            pss.append(ps)
                        for k in range(DK):
                            for n in range(2):
                                nc.tensor.matmul(
                                    pss[n],
                                    wq_s[:, k, m * 128 : (m + 1) * 128],
                                    srcT[:, k, n * 512 : (n + 1) * 512],
                                    start=(k == 0),
                                    stop=(k == DK - 1),
                                )
                        for n in range(2):
                            nc.vector.tensor_scalar_add(
                                out=QT[:, m, n * 512 : (n + 1) * 512],
                                in0=pss[n],
                                scalar1=bq_col[:, (gc0 // 128) + m : (gc0 // 128) + m + 1],
                            )

                    # -- V_g [S2, HPG*(HD+1)] bf16, ones col per head --
                    V = grp.tile([128, S2 // 128, HPG, HD + 1], bf16, tag="V")
                    nc.vector.memset(V[:, :, :, HD : HD + 1], 1.0)
                    for ms in range(S2 // 128):
                        ps = psum.tile([128, GW], f32, tag="mm", bufs=4)
                        for k in range(DK):
                            nc.tensor.matmul(
                                ps,
                                srcT[:, k, ms * 128 : (ms + 1) * 128],
                                wv_s[:, k, :],
                                start=(k == 0),
                                stop=(k == DK - 1),
                            )
                        nc.vector.tensor_add(
                            out=V[:, ms, :, 0:HD],
                            in0=ps.rearrange("p (h d) -> p h d", h=HPG),
                            in1=bv_bc.rearrange("p (h d) -> p h d", h=H)[
                                :, HPG * g : HPG * (g + 1), :
                            ],
                        )

                    # -- attention per head / sq tile --
                    for hh in range(HPG):
                        m_h = hh // 2
                        p0 = (hh % 2) * 64
                        for sq in range(SQ // 512):
                            expS = expp.tile([128, S2 // 128, 512], bf16, tag="expS")
                            for sk in range(S2 // 128):
                                ps = psum.tile([128, 512], f32, tag="sc", bufs=2)
                                nc.tensor.matmul(
                                    ps,
                                    KT[p0 : p0 + 64, m_h, sk * 128 : (sk + 1) * 128],
                                    QT[p0 : p0 + 64, m_h, sq * 512 : (sq + 1) * 512],
                                    start=True,
                                    stop=True,
                                )
                                nc.scalar.activation(
                                    out=expS[:, sk, :], in_=ps, func=AF.Exp, scale=SCALE
                                )
                            pv = psum.tile([HD + 1, 512], f32, tag="pv", bufs=2)
                            for sk in range(S2 // 128):
                                nc.tensor.matmul(
                                    pv,
                                    V[:, sk, hh, :],
                                    expS[:, sk, :],
                                    start=(sk == 0),
                                    stop=(sk == S2 // 128 - 1),
                                )
                            den = nrm.tile([1, 512], f32, tag="den")
                            nc.vector.tensor_copy(out=den, in_=pv[HD : HD + 1, :])
                            den_bc = nrm.tile([64, 512], f32, tag="den_bc")
                            nc.gpsimd.partition_broadcast(den_bc, den)
                            nc.vector.reciprocal_approx_fast(out=den_bc, in_=den_bc)
                            xt = nrm.tile([64, 512], f32r, tag="xt")
                            nc.vector.tensor_mul(out=xt, in0=pv[0:HD, :], in1=den_bc)
                            h_abs = g * HPG + hh
                            nc.sync.dma_start(
                                out=xT_dram[
                                    h_abs * HD : (h_abs + 1) * HD,
                                    sq * 512 : (sq + 1) * 512,
                                ],
                                in_=xt,
                            )

            # ============ out-projection + residual + LN1, then FFN ============
            with contextlib.ExitStack() as fo_ctx:
                src1T_pool = fo_ctx.enter_context(tc.tile_pool(name="src1T", bufs=1))
                src1T = src1T_pool.tile([128, DK, SQ], f32r)

                with contextlib.ExitStack() as octx:
                    opool = octx.enter_context(tc.tile_pool(name="oproj", bufs=1))
                    otmp = octx.enter_context(tc.tile_pool(name="otmp", bufs=2))

                    bo_bc = opool.tile([128, D], f32)
                    nc.gpsimd.dma_start(out=bo_bc, in_=bc_ap(bo, D))
                    g1_bc = opool.tile([128, D], f32)
                    nc.gpsimd.dma_start(out=g1_bc, in_=bc_ap(g1, D))
                    b1_bc = opool.tile([128, D], f32)
                    nc.gpsimd.dma_start(out=b1_bc, in_=bc_ap(b1, D))

                    # chunked loads so matmuls start after the first chunk
                    wo_s = opool.tile([128, DK, D], f32r)
                    xts = opool.tile([128, DK, SQ], f32r)
                    for k in range(DK):
                        nc.sync.dma_start(
                            out=xts[:, k, :],
                            in_=xT_dram[k * 128 : (k + 1) * 128, :],
                        )
                        nc.sync.dma_start(
                            out=wo_s[:, k, :],
                            in_=Wo[k * 128 : (k + 1) * 128, :],
                        )

                    r_sb = opool.tile([128, SQ // 128, D], f32)

                    # pass A: all matmuls + residual adds
                    for m in range(SQ // 128):
                        pss = []
                        for n in range(2):
                            ps = psum.tile([128, 512], f32, tag="mm", bufs=4)
                            pss.append(ps)
                        for k in range(DK):
                            for n in range(2):
                                nc.tensor.matmul(
                                    pss[n],
                                    xts[:, k, m * 128 : (m + 1) * 128],
                                    wo_s[:, k, n * 512 : (n + 1) * 512],
                                    start=(k == 0),
                                    stop=(k == DK - 1),
                                )
                        sq_t = otmp.tile([128, D], f32, tag="sq_ld", bufs=3)
                        nc.sync.dma_start(out=sq_t, in_=src_q[m * 128 : (m + 1) * 128, :])
                        for n in range(2):
                            nc.vector.tensor_add(
                                out=r_sb[:, m, n * 512 : (n + 1) * 512],
                                in0=pss[n],
                                in1=sq_t[:, n * 512 : (n + 1) * 512],
                            )

                    # pass B: LN1 + transposes into src1T (+ src1 to DRAM)
                    for m in range(SQ // 128):
                        rrow = r_sb[:, m, :]
                        nc.vector.tensor_add(out=rrow, in0=rrow, in1=bo_bc)
                        s1 = otmp.tile([128, D], f32, tag="s1", bufs=3)
                        layer_norm(rrow, g1_bc, b1_bc, s1, otmp)
                        nc.sync.dma_start(
                            out=src1_dram[m * 128 : (m + 1) * 128, :], in_=s1
                        )
                        for kk in range(2):
                            ps = psum.tile([128, 512], f32, tag="mm", bufs=4)
                            for j in range(4):
                                k = kk * 4 + j
                                nc.tensor.transpose(
                                    ps[:, j * 128 : (j + 1) * 128],
                                    s1[:, k * 128 : (k + 1) * 128],
                                    identity,
                                )
                            for j in range(4):
                                k = kk * 4 + j
                                nc.vector.tensor_copy(
                                    out=src1T[:, k, m * 128 : (m + 1) * 128],
                                    in_=ps[:, j * 128 : (j + 1) * 128],
                                )

                # ============ FFN ============
                with contextlib.ExitStack() as fctx:
                    hpool = fctx.enter_context(tc.tile_pool(name="hpool", bufs=1))
                    hT = hpool.tile([128, PFK, SQ], bf16)       # 8MB
                    w2p = fctx.enter_context(tc.tile_pool(name="w2p", bufs=1))
                    w2bf = w2p.tile([128, PFK, D], bf16)        # 8MB
                    fcts = fctx.enter_context(tc.tile_pool(name="fcts", bufs=1))
                    ftmp = fctx.enter_context(tc.tile_pool(name="ftmp", bufs=2))

                    bf2_bc = fcts.tile([128, D], f32)
                    nc.gpsimd.dma_start(out=bf2_bc, in_=bc_ap(bf2, D))
                    g2_bc = fcts.tile([128, D], f32)
                    nc.gpsimd.dma_start(out=g2_bc, in_=bc_ap(g2, D))
                    b2_bc = fcts.tile([128, D], f32)
                    nc.gpsimd.dma_start(out=b2_bc, in_=bc_ap(b2, D))

                    # W2 load + cast to bf16 (overlaps FFN1)
                    for k in range(PFK):
                        w2_ld = ftmp.tile([128, D], f32, tag="w2_ld", bufs=2)
                        nc.sync.dma_start(out=w2_ld, in_=W2[k * 128 : (k + 1) * 128, :])
                        nc.gpsimd.tensor_copy(out=w2bf[:, k, :], in_=w2_ld)

                    # FFN1: hT[pf, q] = relu(W1^T src1T + bf1), weight stationary
                    for mp in range(PFK):
                        w1_s = ftmp.tile([128, DK, 128], f32r, tag="w1_s", bufs=2)
                        nc.sync.dma_start(
                            out=w1_s,
                            in_=W1.rearrange("(a p) n -> p a n", p=128)[
                                :, :, mp * 128 : (mp + 1) * 128
                            ],
                        )
                        pss = []
                        for sqh in range(2):
                            ps = psum.tile([128, 512], f32, tag="mm", bufs=4)
                            pss.append(ps)
                        for k in range(DK):
                            for sqh in range(2):
                                nc.tensor.matmul(
                                    pss[sqh],
                                    w1_s[:, k, :],
                                    src1T[:, k, sqh * 512 : (sqh + 1) * 512],
                                    start=(k == 0),
                                    stop=(k == DK - 1),
                                )
                        for sqh in range(2):
                            nc.vector.tensor_scalar(
                                out=hT[:, mp, sqh * 512 : (sqh + 1) * 512],
                                in0=pss[sqh],
                                scalar1=bf1_col[:, mp : mp + 1],
                                scalar2=0.0,
                                op0=ALU.add,
                                op1=ALU.max,
                            )

                    # FFN2: per-m row, weight(hT) stationary over 2 n-halves
                    for m in range(SQ // 128):
                        pss = []
                        for n in range(2):
                            ps = psum.tile([128, 512], f32, tag="mm", bufs=4)
                            pss.append(ps)
                        for k in range(PFK):
                            for n in range(2):
                                nc.tensor.matmul(
                                    pss[n],
                                    hT[:, k, m * 128 : (m + 1) * 128],
                                    w2bf[:, k, n * 512 : (n + 1) * 512],
                                    start=(k == 0),
                                    stop=(k == PFK - 1),
                                )
                        s1_t = ftmp.tile([128, D], f32, tag="s1_ld", bufs=2)
                        nc.sync.dma_start(
                            out=s1_t, in_=src1_dram[m * 128 : (m + 1) * 128, :]
                        )
                        rr = ftmp.tile([128, D], f32, tag="rr", bufs=2)
                        for n in range(2):
                            nc.vector.tensor_add(
                                out=rr[:, n * 512 : (n + 1) * 512],
                                in0=pss[n],
                                in1=s1_t[:, n * 512 : (n + 1) * 512],
                            )
                        nc.vector.tensor_add(out=rr, in0=rr, in1=bf2_bc)
                        layer_norm(rr, g2_bc, b2_bc, rr, ftmp)
                        nc.sync.dma_start(out=out[m * 128 : (m + 1) * 128, :], in_=rr)

    nc.compile()
    return nc


def kernel(**inputs):
    from concourse.bass_utils import run_bass_kernel_spmd

    if "nc" not in _CACHE:
        _CACHE["nc"] = _build()
    nc = _CACHE["nc"]

    ins = {k: np.asarray(v, dtype=np.float32) for k, v in inputs.items()}
    src = ins["src"]
    weight_names = [
        "Wq", "Wk", "Wv", "Wo", "W1", "W2",
        "bq", "bk", "bv", "bo", "bf1", "bf2", "g1", "b1", "g2", "b2",
    ]
    weights = {n: np.ascontiguousarray(ins[n]) for n in weight_names}

    in_maps = []
    for c in range(N_CORES):
        b, h = divmod(c, 2)
        m = dict(weights)
        m["src_q"] = np.ascontiguousarray(src[b, h * SQ : (h + 1) * SQ])
        m["src_o"] = np.ascontiguousarray(src[b, (1 - h) * SQ : (2 - h) * SQ])
        in_maps.append(m)

    res = run_bass_kernel_spmd(nc, in_maps, list(range(N_CORES)))

    out = np.empty((4, S2, D), dtype=np.float32)
    for c in range(N_CORES):
        b, h = divmod(c, 2)
        out[b, h * SQ : (h + 1) * SQ] = res.results[c]["out"]
    return out


# revision 16
# speedup vs baseline: 1.3227x; 1.1397x over previous
"""Trainium2 Bass kernel for a transformer encoder layer.

B=4, S=2048, D=1024, H=16 heads (HD=64), PF=4096, fp32 I/O.

Sharding: 8 cores, core c handles batch c//2, query seq-half c%2 (1024
tokens). Each core computes K/V over its batch's full 2048-token sequence
(duplicated within the pair; ~12% extra flops) so no collectives are needed.

All matmuls run in bf16 (weights cast host-side; activations cast at PSUM
eviction), accumulating in fp32 PSUM. Residual/LayerNorm arithmetic stays
fp32. Attention is kept transposed so no P-matrix transpose is needed:
  srcT = src^T; QT/KT = W^T srcT; V = srcT^T Wv (+ per-head ones column)
  S^T = K Q^T (contraction HD=64), expS = exp(S^T/8)
  x^T = V_aug^T expS (contraction Sk) -> divide by denominator row
  attn_out = (x^T)^T Wo + src + bo -> LN1 -> src1, src1T
  h^T = relu(W1^T src1T + bf1); ff = (h^T)^T W2 + bf2; out = LN2(src1 + ff)
"""

import numpy as np

D = 1024
S2 = 2048
SQ = 1024
PF = 4096
H = 16
HD = 64
DK = D // 128
PFK = PF // 128
NG = 4                 # head groups
HPG = H // NG
GW = HPG * HD          # 256 dims per group
GM = GW // 128
SCALE = 1.0 / 8.0
EPS = 1e-5
N_CORES = 8

_CACHE = {}


def _build():
    import concourse.bass as bass
    import concourse.mybir as mybir
    import concourse.tile as tile
    from concourse import bacc
    from concourse.masks import make_identity

    f32 = mybir.dt.float32
    bf16 = mybir.dt.bfloat16
    AF = mybir.ActivationFunctionType
    ALU = mybir.AluOpType

    nc = bacc.Bacc("TRN2", target_bir_lowering=False, debug=False, num_devices=N_CORES)

    def din(name, shape, dt=f32):
        return nc.dram_tensor(name, shape, dt, kind="ExternalInput")

    src_q = din("src_q", [SQ, D])
    src_o = din("src_o", [SQ, D])
    Wq = din("Wq", [D, D], bf16)
    Wk = din("Wk", [D, D], bf16)
    Wv = din("Wv", [D, D], bf16)
    Wo = din("Wo", [D, D], bf16)
    W1 = din("W1", [D, PF], bf16)
    W2 = din("W2", [PF, D], bf16)
    bq = din("bq", [D])
    bk = din("bk", [D])
    bv = din("bv", [D])
    bo = din("bo", [D])
    bf1 = din("bf1", [PF])
    bf2 = din("bf2", [D])
    g1 = din("g1", [D])
    b1 = din("b1", [D])
    g2 = din("g2", [D])
    b2 = din("b2", [D])
    out = nc.dram_tensor("out", [SQ, D], f32, kind="ExternalOutput")

    xT_dram = nc.dram_tensor("xT_scratch", [D, SQ], bf16)
    src1_dram = nc.dram_tensor("src1_scratch", [SQ, D], f32)

    def bc_ap(vec, n):
        return bass.AP(tensor=vec, offset=0, ap=[[0, 128], [1, n]])

    def col_ap(vec, m):
        return bass.AP(tensor=vec, offset=0, ap=[[1, 128], [128, m]])

    with tile.TileContext(nc) as tc:
        import contextlib

        with contextlib.ExitStack() as ctx:
            consts = ctx.enter_context(tc.tile_pool(name="consts", bufs=1))
            psum = ctx.enter_context(tc.tile_pool(name="psum", bufs=1, space="PSUM"))

            identity = consts.tile([128, 128], f32)
            make_identity(nc, identity)

            bq_col = consts.tile([128, DK], f32)
            nc.sync.dma_start(out=bq_col, in_=col_ap(bq, DK))
            bk_col = consts.tile([128, DK], f32)
            nc.sync.dma_start(out=bk_col, in_=col_ap(bk, DK))
            bf1_col = consts.tile([128, PFK], f32)
            nc.sync.dma_start(out=bf1_col, in_=col_ap(bf1, PFK))

            eps_t = consts.tile([128, 1], f32)
            nc.vector.memset(eps_t, EPS)

            def layer_norm(r_row, g_bc, b_bc, out_tile, tmp_pool):
                stats = tmp_pool.tile([128, 2, 6], f32, tag="ln_stats")
                rr = r_row.rearrange("p (a f) -> p a f", a=2)
                for a in range(2):
                    nc.vector.bn_stats(out=stats[:, a, :], in_=rr[:, a, :])
                mv = tmp_pool.tile([128, 2], f32, tag="ln_mv")
                nc.vector.bn_aggr(out=mv, in_=stats)
                rstd = tmp_pool.tile([128, 1], f32, tag="ln_rstd")
                nc.scalar.activation(
                    out=rstd, in_=mv[:, 1:2], func=AF.Sqrt, bias=eps_t, scale=1.0
                )
                nc.vector.reciprocal_approx_fast(out=rstd, in_=rstd)
                nc.vector.tensor_scalar(
                    out=out_tile,
                    in0=r_row,
                    scalar1=mv[:, 0:1],
                    scalar2=rstd,
                    op0=ALU.subtract,
                    op1=ALU.mult,
                )
                nc.vector.tensor_mul(out=out_tile, in0=out_tile, in1=g_bc)
                nc.vector.tensor_add(out=out_tile, in0=out_tile, in1=b_bc)

            # ============ Phase 0 + attention ============
            with contextlib.ExitStack() as attn_ctx:
                srctp = attn_ctx.enter_context(tc.tile_pool(name="srctp", bufs=1))
                srcT = srctp.tile([128, DK, S2], bf16)  # 4MB

                with tc.tile_pool(name="ph0", bufs=2) as ph0:
                    for blk in range(4):
                        half = src_q if blk < 2 else src_o
                        base = blk * 512
                        sts = []
                        for j in range(4):
                            st = ph0.tile([128, D], f32, tag="src_ld", bufs=6)
                            row0 = (blk % 2) * 512 + j * 128
                            nc.sync.dma_start(out=st, in_=half[row0 : row0 + 128, :])
                            sts.append(st)
                        for k in range(DK):
                            ps = psum.tile([128, 512], f32, tag="big", bufs=3)
                            for j in range(4):
                                nc.tensor.transpose(
                                    ps[:, j * 128 : (j + 1) * 128],
                                    sts[j][:, k * 128 : (k + 1) * 128],
                                    identity,
                                )
                            nc.vector.tensor_copy(
                                out=srcT[:, k, base : base + 512], in_=ps
                            )

                acts = attn_ctx.enter_context(tc.tile_pool(name="acts", bufs=1))
                bv_bc = acts.tile([128, D], f32)
                nc.gpsimd.dma_start(out=bv_bc, in_=bc_ap(bv, D))

                grp = attn_ctx.enter_context(tc.tile_pool(name="grp", bufs=2))
                wst = attn_ctx.enter_context(tc.tile_pool(name="wst", bufs=2))
                expp = attn_ctx.enter_context(tc.tile_pool(name="expp", bufs=2))
                nrm = attn_ctx.enter_context(tc.tile_pool(name="nrm", bufs=2))

                for g in range(NG):
                    gc0 = g * GW

                    wk_s = wst.tile([128, DK, GW], bf16, tag="wk_s")
                    nc.sync.dma_start(
                        out=wk_s,
                        in_=Wk.rearrange("(a p) n -> p a n", p=128)[:, :, gc0 : gc0 + GW],
                    )
                    wq_s = wst.tile([128, DK, GW], bf16, tag="wq_s")
                    nc.sync.dma_start(
                        out=wq_s,
                        in_=Wq.rearrange("(a p) n -> p a n", p=128)[:, :, gc0 : gc0 + GW],
                    )
                    wv_s = wst.tile([128, DK, GW], bf16, tag="wv_s")
                    nc.sync.dma_start(
                        out=wv_s,
                        in_=Wv.rearrange("(a p) n -> p a n", p=128)[:, :, gc0 : gc0 + GW],
                    )

                    # -- KT_g [GW, S2] bf16: 2 blocks of 1024, weight held per k --
                    KT = grp.tile([128, GM, S2], bf16, tag="KT")
                    for m in range(GM):
                        for nn in range(S2 // 1024):
                            ps = psum.tile([128, 1024], f32, tag="big", bufs=3)
                            for k in range(DK):
                                for hf in range(2):
                                    nc.tensor.matmul(
                                        ps[:, hf * 512 : (hf + 1) * 512],
                                        wk_s[:, k, m * 128 : (m + 1) * 128],
                                        srcT[
                                            :,
                                            k,
                                            nn * 1024 + hf * 512 : nn * 1024
                                            + (hf + 1) * 512,
                                        ],
                                        start=(k == 0),
                                        stop=(k == DK - 1),
                                    )
                            nc.vector.tensor_scalar_add(
                                out=KT[:, m, nn * 1024 : (nn + 1) * 1024],
                                in0=ps,
                                scalar1=bk_col[
                                    :, (gc0 // 128) + m : (gc0 // 128) + m + 1
                                ],
                            )

                    # -- QT_g [GW, SQ] bf16 --
                    QT = grp.tile([128, GM, SQ], bf16, tag="QT")
                    for m in range(GM):
                        ps = psum.tile([128, 1024], f32, tag="big", bufs=3)
                        for k in range(DK):
                            for hf in range(2):
                                nc.tensor.matmul(
                                    ps[:, hf * 512 : (hf + 1) * 512],
                                    wq_s[:, k, m * 128 : (m + 1) * 128],
                                    srcT[:, k, hf * 512 : (hf + 1) * 512],
                                    start=(k == 0),
                                    stop=(k == DK - 1),
                                )
                        nc.vector.tensor_scalar_add(
                            out=QT[:, m, :],
                            in0=ps,
                            scalar1=bq_col[:, (gc0 // 128) + m : (gc0 // 128) + m + 1],
                        )

                    # -- V_g [S2, HPG*(HD+1)] bf16 --
                    V = grp.tile([128, S2 // 128, HPG, HD + 1], bf16, tag="V")
                    nc.vector.memset(V[:, :, :, HD : HD + 1], 1.0)
                    for ms in range(S2 // 128):
                        ps = psum.tile([128, GW], f32, tag="big", bufs=3)
                        for k in range(DK):
                            nc.tensor.matmul(
                                ps,
                                srcT[:, k, ms * 128 : (ms + 1) * 128],
                                wv_s[:, k, :],
                                start=(k == 0),
                                stop=(k == DK - 1),
                            )
                        nc.vector.tensor_add(
                            out=V[:, ms, :, 0:HD],
                            in0=ps.rearrange("p (h d) -> p h d", h=HPG),
                            in1=bv_bc.rearrange("p (h d) -> p h d", h=H)[
                                :, HPG * g : HPG * (g + 1), :
                            ],
                        )

                    # -- attention: per head, both sq halves share each psum --
                    for hh in range(HPG):
                        m_h = hh // 2
                        p0 = (hh % 2) * 64
                        expS = expp.tile([128, S2 // 128, SQ], bf16, tag="expS")
                        for sk in range(S2 // 128):
                            ps = psum.tile([128, 1024], f32, tag="big", bufs=3)
                            for sq in range(2):
                                nc.tensor.matmul(
                                    ps[:, sq * 512 : (sq + 1) * 512],
                                    KT[p0 : p0 + 64, m_h, sk * 128 : (sk + 1) * 128],
                                    QT[p0 : p0 + 64, m_h, sq * 512 : (sq + 1) * 512],
                                    start=True,
                                    stop=True,
                                )
                            nc.scalar.activation(
                                out=expS[:, sk, :], in_=ps, func=AF.Exp, scale=SCALE
                            )
                        pv = psum.tile([HD + 1, SQ], f32, tag="pv", bufs=1)
                        for sk in range(S2 // 128):
                            for sq in range(2):
                                nc.tensor.matmul(
                                    pv[:, sq * 512 : (sq + 1) * 512],
                                    V[:, sk, hh, :],
                                    expS[:, sk, sq * 512 : (sq + 1) * 512],
                                    start=(sk == 0),
                                    stop=(sk == S2 // 128 - 1),
                                )
                        den = nrm.tile([1, SQ], f32, tag="den")
                        nc.vector.tensor_copy(out=den, in_=pv[HD : HD + 1, :])
                        den_bc = nrm.tile([64, SQ], f32, tag="den_bc")
                        nc.gpsimd.partition_broadcast(den_bc, den)
                        nc.vector.reciprocal_approx_fast(out=den_bc, in_=den_bc)
                        xt = nrm.tile([64, SQ], bf16, tag="xt")
                        nc.vector.tensor_mul(out=xt, in0=pv[0:HD, :], in1=den_bc)
                        h_abs = g * HPG + hh
                        nc.sync.dma_start(
                            out=xT_dram[h_abs * HD : (h_abs + 1) * HD, :], in_=xt
                        )

            # ============ out-projection + LN1 + FFN ============
            with contextlib.ExitStack() as fo_ctx:
                src1T_pool = fo_ctx.enter_context(tc.tile_pool(name="src1T", bufs=1))
                src1T = src1T_pool.tile([128, DK, SQ], bf16)

                with contextlib.ExitStack() as octx:
                    opool = octx.enter_context(tc.tile_pool(name="oproj", bufs=1))
                    otmp = octx.enter_context(tc.tile_pool(name="otmp", bufs=2))

                    bo_bc = opool.tile([128, D], f32)
                    nc.gpsimd.dma_start(out=bo_bc, in_=bc_ap(bo, D))
                    g1_bc = opool.tile([128, D], f32)
                    nc.gpsimd.dma_start(out=g1_bc, in_=bc_ap(g1, D))
                    b1_bc = opool.tile([128, D], f32)
                    nc.gpsimd.dma_start(out=b1_bc, in_=bc_ap(b1, D))

                    wo_s = opool.tile([128, DK, D], bf16)
                    xts = opool.tile([128, DK, SQ], bf16)
                    for k in range(DK):
                        nc.sync.dma_start(
                            out=xts[:, k, :], in_=xT_dram[k * 128 : (k + 1) * 128, :]
                        )
                        nc.sync.dma_start(
                            out=wo_s[:, k, :], in_=Wo[k * 128 : (k + 1) * 128, :]
                        )

                    r_sb = opool.tile([128, SQ // 128, D], f32)

                    for m in range(SQ // 128):
                        ps = psum.tile([128, 1024], f32, tag="big", bufs=3)
                        for k in range(DK):
                            for n in range(2):
                                nc.tensor.matmul(
                                    ps[:, n * 512 : (n + 1) * 512],
                                    xts[:, k, m * 128 : (m + 1) * 128],
                                    wo_s[:, k, n * 512 : (n + 1) * 512],
                                    start=(k == 0),
                                    stop=(k == DK - 1),
                                )
                        sq_t = otmp.tile([128, D], f32, tag="sq_ld", bufs=3)
                        nc.sync.dma_start(out=sq_t, in_=src_q[m * 128 : (m + 1) * 128, :])
                        nc.vector.tensor_add(out=r_sb[:, m, :], in0=ps, in1=sq_t)

                    for m in range(SQ // 128):
                        rrow = r_sb[:, m, :]
                        nc.vector.tensor_add(out=rrow, in0=rrow, in1=bo_bc)
                        s1 = otmp.tile([128, D], f32, tag="s1", bufs=3)
                        layer_norm(rrow, g1_bc, b1_bc, s1, otmp)
                        nc.sync.dma_start(
                            out=src1_dram[m * 128 : (m + 1) * 128, :], in_=s1
                        )
                        for kk in range(2):
                            ps = psum.tile([128, 512], f32, tag="big", bufs=3)
                            for j in range(4):
                                k = kk * 4 + j
                                nc.tensor.transpose(
                                    ps[:, j * 128 : (j + 1) * 128],
                                    s1[:, k * 128 : (k + 1) * 128],
                                    identity,
                                )
                            for j in range(4):
                                k = kk * 4 + j
                                nc.vector.tensor_copy(
                                    out=src1T[:, k, m * 128 : (m + 1) * 128],
                                    in_=ps[:, j * 128 : (j + 1) * 128],
                                )

                # ============ FFN ============
                with contextlib.ExitStack() as fctx:
                    hpool = fctx.enter_context(tc.tile_pool(name="hpool", bufs=1))
                    hT = hpool.tile([128, PFK, SQ], bf16)     # 8MB
                    w2p = fctx.enter_context(tc.tile_pool(name="w2p", bufs=1))
                    w2bf = w2p.tile([128, PFK, D], bf16)      # 8MB
                    fcts = fctx.enter_context(tc.tile_pool(name="fcts", bufs=1))
                    ftmp = fctx.enter_context(tc.tile_pool(name="ftmp", bufs=2))

                    bf2_bc = fcts.tile([128, D], f32)
                    nc.gpsimd.dma_start(out=bf2_bc, in_=bc_ap(bf2, D))
                    g2_bc = fcts.tile([128, D], f32)
                    nc.gpsimd.dma_start(out=g2_bc, in_=bc_ap(g2, D))
                    b2_bc = fcts.tile([128, D], f32)
                    nc.gpsimd.dma_start(out=b2_bc, in_=bc_ap(b2, D))

                    nc.sync.dma_start(
                        out=w2bf, in_=W2.rearrange("(a p) n -> p a n", p=128)
                    )

                    # FFN1: hT[pf, q] = relu(W1^T src1T + bf1)
                    for mp in range(PFK):
                        w1_s = ftmp.tile([128, DK, 128], bf16, tag="w1_s", bufs=3)
                        nc.sync.dma_start(
                            out=w1_s,
                            in_=W1.rearrange("(a p) n -> p a n", p=128)[
                                :, :, mp * 128 : (mp + 1) * 128
                            ],
                        )
                        ps = psum.tile([128, 1024], f32, tag="big", bufs=3)
                        for k in range(DK):
                            for sqh in range(2):
                                nc.tensor.matmul(
                                    ps[:, sqh * 512 : (sqh + 1) * 512],
                                    w1_s[:, k, :],
                                    src1T[:, k, sqh * 512 : (sqh + 1) * 512],
                                    start=(k == 0),
                                    stop=(k == DK - 1),
                                )
                        nc.vector.tensor_scalar(
                            out=hT[:, mp, :],
                            in0=ps,
                            scalar1=bf1_col[:, mp : mp + 1],
                            scalar2=0.0,
                            op0=ALU.add,
                            op1=ALU.max,
                        )

                    # FFN2 per m row + residual + LN2
                    for m in range(SQ // 128):
                        ps = psum.tile([128, 1024], f32, tag="big", bufs=3)
                        for k in range(PFK):
                            for n in range(2):
                                nc.tensor.matmul(
                                    ps[:, n * 512 : (n + 1) * 512],
                                    hT[:, k, m * 128 : (m + 1) * 128],
                                    w2bf[:, k, n * 512 : (n + 1) * 512],
                                    start=(k == 0),
                                    stop=(k == PFK - 1),
                                )
                        s1_t = ftmp.tile([128, D], f32, tag="s1_ld", bufs=2)
                        nc.sync.dma_start(
                            out=s1_t, in_=src1_dram[m * 128 : (m + 1) * 128, :]
                        )
                        rr = ftmp.tile([128, D], f32, tag="rr", bufs=2)
                        nc.vector.tensor_add(out=rr, in0=ps, in1=s1_t)
                        nc.vector.tensor_add(out=rr, in0=rr, in1=bf2_bc)
                        layer_norm(rr, g2_bc, b2_bc, rr, ftmp)
                        nc.sync.dma_start(out=out[m * 128 : (m + 1) * 128, :], in_=rr)

    nc.compile()
    return nc


def kernel(**inputs):
    import ml_dtypes
    from concourse.bass_utils import run_bass_kernel_spmd

    if "nc" not in _CACHE:
        _CACHE["nc"] = _build()
    nc = _CACHE["nc"]

    ins = {k: np.asarray(v, dtype=np.float32) for k, v in inputs.items()}
    src = ins["src"]
    bf = ml_dtypes.bfloat16
    weights = {}
    for n in ["Wq", "Wk", "Wv", "Wo", "W1", "W2"]:
        weights[n] = np.ascontiguousarray(ins[n]).astype(bf)
    for n in ["bq", "bk", "bv", "bo", "bf1", "bf2", "g1", "b1", "g2", "b2"]:
        weights[n] = np.ascontiguousarray(ins[n])

    in_maps = []
    for c in range(N_CORES):
        b, h = divmod(c, 2)
        m = dict(weights)
        m["src_q"] = np.ascontiguousarray(src[b, h * SQ : (h + 1) * SQ])
        m["src_o"] = np.ascontiguousarray(src[b, (1 - h) * SQ : (2 - h) * SQ])
        in_maps.append(m)

    res = run_bass_kernel_spmd(nc, in_maps, list(range(N_CORES)))

    out = np.empty((4, S2, D), dtype=np.float32)
    for c in range(N_CORES):
        b, h = divmod(c, 2)
        out[b, h * SQ : (h + 1) * SQ] = res.results[c]["out"]
    return out
